# Initial kernel scaffold
#
"""DiceLoss Trainium2 kernel: softmax dice loss over [4,150,512,512] logits.

Sharding: pure data parallel over pixels. Core k handles half of batch k//2
(131072 contiguous pixels per class row). Each core computes per-class partial
sums (intersection, x_sum, y_sum); host sums the 8 cores' partials and forms
the dice ratio (tiny [150]-vector epilogue).

Layout on device (natural, class-major):
  - 64 slabs of 2048 pixels; classes 0..127 as [128,2048] tiles, classes
    128..149 packed 5-slabs-up into [110,2048] tiles (plus one [22,2048]
    leftover per 16-slab phase).
  - ACT: exp(x) -> bf16. PE: per-pixel softmax denominator Z via indicator
    matmuls into PSUM [16,2048] (one row per slab). DVE: 1/Z (Newton approx),
    then fused tensor_scalar / tensor_tensor_reduce produce per-class
    partials. GPSIMD: p = E * W elementwise. DMA replicates the per-pixel
    W row and bf16 target row across partitions.
"""

import os
import sys

import numpy as np

for _p in ("/opt/trn_rl_repo", os.path.expanduser("~/.axon_site/_ro/trn_rl_repo")):
    if os.path.isdir(_p) and _p not in sys.path:
        sys.path.insert(0, _p)

import ml_dtypes

from contextlib import ExitStack

import concourse.bass as bass
import concourse.tile as tile
from concourse import mybir

F32 = mybir.dt.float32
BF16 = mybir.dt.bfloat16
AF = mybir.ActivationFunctionType
OP = mybir.AluOpType

NUM_CLASSES = 150
SMOOTH = 1e-05
N_CORES = 8
PIX_CORE = 131072        # pixels per core (4*512*512 / 8)
SLAB = 2048              # pixels per slab
NSLAB = PIX_CORE // SLAB  # 64
PHASE_SLABS = 16
NPHASE = NSLAB // PHASE_SLABS  # 4
C0 = 128                 # classes in chunk0
C1 = NUM_CLASSES - C0    # 22 classes in chunk1
GRP = 5                  # slabs packed per chunk1 tile (5*22=110 partitions)

# accumulator column layout in the output tensor [128, 240]
#   cols   0- 63 : intersection partials, classes 0..127, one col per slab
#   cols  64-127 : x_sum partials, classes 0..127
#   cols 128-191 : y_sum partials, classes 0..127
#   cols 192-207 : intersection partials, chunk1 (rows 22j+c or 0..21)
#   cols 208-223 : x_sum partials, chunk1
#   cols 224-239 : y_sum partials, chunk1
OUT_COLS = 240


def _build_consts():
    # selmat: for slab-row r in [0,16): lhsT [128,16] with column r all-ones
    selmat = np.zeros((128, 256), dtype=ml_dtypes.bfloat16)
    for r in range(16):
        selmat[:, r * 16 + r] = 1
    # blockdiag for chunk1 groups g'=0..2: [110,16], row p -> col 5g'+p//22
    bdiag = np.zeros((128, 48), dtype=ml_dtypes.bfloat16)
    for g in range(3):
        for p in range(110):
            bdiag[p, g * 16 + (5 * g + p // 22)] = 1
    # single leftover slab (phase row 15): [22,16] col 15
    single = np.zeros((128, 16), dtype=ml_dtypes.bfloat16)
    for p in range(C1):
        single[p, 15] = 1
    cmat = np.concatenate([selmat, bdiag, single], axis=1)  # [128, 320]

    iotas = np.zeros((128, 2), dtype=np.float32)
    iotas[:, 0] = np.arange(128)
    iotas[:110, 1] = C0 + (np.arange(110) % C1)
    return cmat, iotas


def build_nc():
    nc = bass.Bass()
    x = nc.dram_tensor("x", [NUM_CLASSES, PIX_CORE], F32, kind="ExternalInput")
    tb = nc.dram_tensor("tb", [NSLAB, SLAB], BF16, kind="ExternalInput")
    cmat = nc.dram_tensor("cmat", [128, 320], BF16, kind="ExternalInput")
    iotas = nc.dram_tensor("iotas", [128, 2], F32, kind="ExternalInput")
    out = nc.dram_tensor("out", [128, OUT_COLS], F32, kind="ExternalOutput")

    with ExitStack() as ctx, tile.TileContext(nc) as tc:
        const_p = ctx.enter_context(tc.tile_pool(name="const", bufs=1))
        x0_p = ctx.enter_context(tc.tile_pool(name="x0", bufs=2))
        x1_p = ctx.enter_context(tc.tile_pool(name="x1", bufs=2))
        e0_p = ctx.enter_context(tc.tile_pool(name="e0", bufs=16))
        e1_p = ctx.enter_context(tc.tile_pool(name="e1", bufs=4))
        tb_p = ctx.enter_context(tc.tile_pool(name="tbp", bufs=2))
        wf_p = ctx.enter_context(tc.tile_pool(name="wf", bufs=1))
        wb_p = ctx.enter_context(tc.tile_pool(name="wb", bufs=2))
        bc_p = ctx.enter_context(tc.tile_pool(name="bc", bufs=2))
        mask_p = ctx.enter_context(tc.tile_pool(name="mask", bufs=3))
        pp_p = ctx.enter_context(tc.tile_pool(name="pp", bufs=3))
        scr_p = ctx.enter_context(tc.tile_pool(name="scr", bufs=1))
        acc_p = ctx.enter_context(tc.tile_pool(name="acc", bufs=1))
        zp_p = ctx.enter_context(
            tc.tile_pool(name="zp", bufs=2, space=bass.MemorySpace.PSUM)
        )

        cm = const_p.tile([128, 320], BF16, tag="cm")
        nc.sync.dma_start(cm[:], cmat[:])
        io = const_p.tile([128, 2], F32, tag="io")
        nc.sync.dma_start(io[:], iotas[:])

        is0 = acc_p.tile([128, NSLAB], F32, tag="is0")
        xs0 = acc_p.tile([128, NSLAB], F32, tag="xs0")
        ys0 = acc_p.tile([128, NSLAB], F32, tag="ys0")
        is1 = acc_p.tile([110, 16], F32, tag="is1")
        xs1 = acc_p.tile([110, 16], F32, tag="xs1")
        ys1 = acc_p.tile([110, 16], F32, tag="ys1")

        for ph in range(NPHASE):
            zp = zp_p.tile([PHASE_SLABS, SLAB], F32, tag="zp")

            # ---- stage A: load, exp, Z accumulation ----
            e0s = []
            for r in range(PHASE_SLABS):
                s = ph * PHASE_SLABS + r
                x0 = x0_p.tile([128, SLAB], F32, tag="x0")
                nc.sync.dma_start(x0[:], x[0:128, s * SLAB:(s + 1) * SLAB])
                e0 = e0_p.tile([128, SLAB], BF16, tag="e0")
                nc.scalar.activation(e0[:], x0[:], AF.Exp)
                e0s.append(e0)
                lhsT = cm[:, r * 16:(r + 1) * 16]
                for j in range(4):
                    nc.tensor.matmul(
                        zp[:, j * 512:(j + 1) * 512],
                        lhsT,
                        e0[:, j * 512:(j + 1) * 512],
                        start=(r == 0),
                        stop=False,
                    )

            e1s = []
            for g in range(3):
                x1 = x1_p.tile([110, SLAB], F32, tag="x1")
                for j in range(GRP):
                    s = ph * PHASE_SLABS + g * GRP + j
                    nc.sync.dma_start(
                        x1[22 * j:22 * j + 22, :],
                        x[C0:NUM_CLASSES, s * SLAB:(s + 1) * SLAB],
                    )
                e1 = e1_p.tile([110, SLAB], BF16, tag="e1")
                nc.scalar.activation(e1[:], x1[:], AF.Exp)
                e1s.append(e1)
                lhsT = cm[0:110, 256 + g * 16:256 + (g + 1) * 16]
                for j in range(4):
                    nc.tensor.matmul(
                        zp[:, j * 512:(j + 1) * 512],
                        lhsT,
                        e1[:, j * 512:(j + 1) * 512],
                        start=False,
                        stop=False,
                    )
            # leftover slab (phase row 15)
            s15 = ph * PHASE_SLABS + 15
            x1s = x1_p.tile([C1, SLAB], F32, tag="x1")
            nc.sync.dma_start(
                x1s[:], x[C0:NUM_CLASSES, s15 * SLAB:(s15 + 1) * SLAB]
            )
            e1x = e1_p.tile([C1, SLAB], BF16, tag="e1")
            nc.scalar.activation(e1x[:], x1s[:], AF.Exp)
            lhsT = cm[0:C1, 304:320]
            for j in range(4):
                nc.tensor.matmul(
                    zp[:, j * 512:(j + 1) * 512],
                    lhsT,
                    e1x[:, j * 512:(j + 1) * 512],
                    start=False,
                    stop=(j == 3),
                )

            # ---- stage W: per-pixel reciprocal of Z, targets row ----
            tbp = tb_p.tile([PHASE_SLABS, SLAB], BF16, tag="tbp")
            nc.sync.dma_start(
                tbp[:], tb[ph * PHASE_SLABS:(ph + 1) * PHASE_SLABS, :]
            )
            wf = wf_p.tile([PHASE_SLABS, SLAB], F32, tag="wf")
            wscr = wf_p.tile([PHASE_SLABS, SLAB], F32, tag="wscr")
            nc.vector.reciprocal_approx_accurate(out=wf[:], in_=zp[:], scratch=wscr[:])
            wb = wb_p.tile([PHASE_SLABS, SLAB], BF16, tag="wb")
            nc.vector.tensor_copy(wb[:], wf[:])

            # ---- stage B: per-class partials ----
            def stageB(e_t, parts, tcol, wcol, iota_ap, acc_i, acc_x, acc_y, col):
                mask = mask_p.tile([128, SLAB], BF16, tag="mask")
                m = mask[0:parts, :]
                nc.vector.tensor_scalar(
                    m, tcol, iota_ap, None, OP.is_equal,
                    accum_out=acc_y[0:parts, col:col + 1],
                )
                p = pp_p.tile([128, SLAB], BF16, tag="pp")
                pt = p[0:parts, :]
                nc.gpsimd.tensor_tensor(pt, e_t, wcol, OP.mult)
                scr = scr_p.tile([128, SLAB], BF16, tag="scr")
                nc.vector.tensor_tensor_reduce(
                    out=scr[0:parts, :], in0=pt, in1=m, scale=1.0, scalar=0.0,
                    op0=OP.mult, op1=OP.add,
                    accum_out=acc_i[0:parts, col:col + 1],
                )
                scr2 = scr_p.tile([128, SLAB], BF16, tag="scr")
                nc.vector.tensor_tensor_reduce(
                    out=scr2[0:parts, :], in0=pt, in1=pt, scale=1.0, scalar=0.0,
                    op0=OP.mult, op1=OP.add,
                    accum_out=acc_x[0:parts, col:col + 1],
                )

            for r in range(PHASE_SLABS):
                s = ph * PHASE_SLABS + r
                tbc = bc_p.tile([128, SLAB], BF16, tag="tbc")
                nc.sync.dma_start(tbc[:], tbp[r:r + 1, :].partition_broadcast(128))
                wbc = bc_p.tile([128, SLAB], BF16, tag="wbc")
                nc.sync.dma_start(wbc[:], wb[r:r + 1, :].partition_broadcast(128))
                stageB(e0s[r][:], 128, tbc[:], wbc[:], io[:, 0:1],
                       is0, xs0, ys0, s)

            for g in range(4):
                parts = 110 if g < 3 else C1
                nsl = GRP if g < 3 else 1
                tbc = bc_p.tile([128, SLAB], BF16, tag="tbc")
                wbc = bc_p.tile([128, SLAB], BF16, tag="wbc")
                for j in range(nsl):
                    r = g * GRP + j
                    nc.sync.dma_start(
                        tbc[22 * j:22 * j + 22, :],
                        tbp[r:r + 1, :].partition_broadcast(22),
                    )
                    nc.sync.dma_start(
                        wbc[22 * j:22 * j + 22, :],
                        wb[r:r + 1, :].partition_broadcast(22),
                    )
                e_t = (e1s[g][:] if g < 3 else e1x[:])
                stageB(e_t, parts, tbc[0:parts, :], wbc[0:parts, :],
                       io[0:parts, 1:2], is1, xs1, ys1, ph * 4 + g)

        nc.sync.dma_start(out[:, 0:64], is0[:])
        nc.sync.dma_start(out[:, 64:128], xs0[:])
        nc.sync.dma_start(out[:, 128:192], ys0[:])
        nc.sync.dma_start(out[0:110, 192:208], is1[:])
        nc.sync.dma_start(out[0:110, 208:224], xs1[:])
        nc.sync.dma_start(out[0:110, 224:240], ys1[:])

    return nc


def make_in_maps(inputs, targets):
    x = np.asarray(inputs, dtype=np.float32).reshape(4, NUM_CLASSES, 512 * 512)
    t = np.asarray(targets)
    t = np.where(t == 255, 0, t).reshape(4, 512 * 512)
    cmat, iotas = _build_consts()
    in_maps = []
    for k in range(N_CORES):
        b, po = k // 2, (k % 2) * PIX_CORE
        xk = np.ascontiguousarray(x[b, :, po:po + PIX_CORE])
        tk = t[b, po:po + PIX_CORE].astype(ml_dtypes.bfloat16).reshape(NSLAB, SLAB)
        in_maps.append({"x": xk, "tb": tk, "cmat": cmat, "iotas": iotas})
    return in_maps


def decode_outputs(outs):
    """outs: list of 8 [128, 240] f32 arrays -> (loss, dice)."""
    IS = np.zeros(NUM_CLASSES, dtype=np.float64)
    XS = np.zeros(NUM_CLASSES, dtype=np.float64)
    YS = np.zeros(NUM_CLASSES, dtype=np.float64)
    cols_g = [c for c in range(16) if c % 4 != 3]
    cols_s = [c for c in range(16) if c % 4 == 3]
    for o in outs:
        o = np.asarray(o, dtype=np.float64)
        IS[:C0] += o[:, 0:64].sum(axis=1)
        XS[:C0] += o[:, 64:128].sum(axis=1)
        YS[:C0] += o[:, 128:192].sum(axis=1)
        for dst, base in ((IS, 192), (XS, 208), (YS, 224)):
            blk = o[0:110, base:base + 16]
            dst[C0:] += blk[:, cols_g].reshape(GRP, C1, len(cols_g)).sum(axis=(0, 2))
            dst[C0:] += blk[0:C1, cols_s].sum(axis=1)
    dice = (2.0 * IS + SMOOTH) / (XS + YS + SMOOTH)
    loss = np.mean(1.0 - dice)
    return np.float32(loss), dice.astype(np.float32)


_NC_CACHE = {}


def _get_nc():
    if "nc" not in _NC_CACHE:
        _NC_CACHE["nc"] = build_nc()
    return _NC_CACHE["nc"]


def kernel(inputs, targets):
    from concourse.bass_utils import run_bass_kernel_spmd

    nc = _get_nc()
    in_maps = make_in_maps(inputs, targets)
    res = run_bass_kernel_spmd(nc, in_maps, list(range(N_CORES)))
    outs = [res.results[i]["out"] for i in range(N_CORES)]
    return decode_outputs(outs)


# revision 17
# speedup vs baseline: 1.2631x; 1.2631x over previous
"""DiceLoss Trainium2 kernel: softmax dice loss over [4,150,512,512] logits.

Sharding: pure data parallel over pixels. Core k handles half of batch k//2
(131072 contiguous pixels per class row). Each core computes per-class partial
sums (intersection, x_sum, y_sum); host sums the 8 cores' partials and forms
the dice ratio (tiny [150]-vector epilogue).

Device layout (natural, class-major), per core:
  - 128 slabs of 1024 pixels, processed in 8 phases of 16 slabs (8 pairs).
  - chunk0 = classes 0..127 as [128, 2048] pair tiles; chunk1 = classes
    128..149 packed 5-slabs-up into [110, 1024] tiles (plus one [22, 1024]
    leftover per phase).
  - ACT: exp(x)->bf16 and the x_sum Square+accumulate. PE: per-pixel softmax
    denominator Z via indicator matmuls into PSUM [16,1024]. DVE: 1/Z
    (Newton approx), per-class masks (tensor_scalar is_equal + accumulate
    = y_sum), p = E*W, p*mask + accumulate = intersection. DMA replicates
    the per-pixel [T|W] rows across partitions (zero-stride inner dim).
  All DMAs are issued from gpsimd (SWDGE) because HWDGE dynamic DMAs only
  support a single sync-wait command.
"""

import os
import sys

import numpy as np

for _p in ("/opt/trn_rl_repo", os.path.expanduser("~/.axon_site/_ro/trn_rl_repo")):
    if os.path.isdir(_p) and _p not in sys.path:
        sys.path.insert(0, _p)

import dataclasses

import ml_dtypes

from contextlib import ExitStack

import concourse.bass as bass
import concourse.tile as tile
from concourse import bacc, mybir

F32 = mybir.dt.float32
BF16 = mybir.dt.bfloat16
AF = mybir.ActivationFunctionType
OP = mybir.AluOpType

NUM_CLASSES = 150
SMOOTH = 1e-05
N_CORES = 8
PIX_CORE = 131072        # pixels per core (4*512*512 / 8)
SLAB = 1024              # pixels per slab
NSLAB = PIX_CORE // SLAB  # 128
PHASE_SLABS = 16
NPHASE = NSLAB // PHASE_SLABS  # 8
NPAIR = PHASE_SLABS // 2  # 8 slab-pairs per phase
C0 = 128                 # classes in chunk0
C1 = NUM_CLASSES - C0    # 22 classes in chunk1
GRP = 5                  # slabs packed per chunk1 tile (5*22=110 partitions)

N_P_COLS = NPHASE * NPAIR       # 64 chunk0 accumulator columns
N_C1_COLS = NPHASE * 4          # 32 chunk1 accumulator columns
OUT_COLS = 3 * N_P_COLS + 3 * N_C1_COLS  # 288


def _ap(src_ap, dims, extra_offset=0):
    """Raw-AP override: replace the dim list (and bump offset) of an AP."""
    return dataclasses.replace(
        src_ap, ap=dims, offset=src_ap.offset + extra_offset
    )


def _bcast_src(row_ap, n):
    """AP replicating a [1, N] SBUF row across n partitions for DMA.

    partition_broadcast puts the zero-stride dim first, which DMA lowering
    rejects; reorder so dim0 is the (trivial) nonzero-stride row dim.
    """
    b = row_ap.partition_broadcast(n)
    return dataclasses.replace(b, ap=[b.ap[1], b.ap[0], b.ap[2]])


def _build_consts():
    # Z lives in PSUM as [8, 2048]: slab r -> row r//2, col-half r%2.
    # sel8 variant q: [128, 8] with column q all-ones
    sel8 = np.zeros((128, 64), dtype=ml_dtypes.bfloat16)
    for q in range(8):
        sel8[:, q * 8 + q] = 1
    # chunk1 blockdiag variants (g, parity): [110, 8],
    # row 22j+c -> col (5g+j)//2 iff (5g+j) % 2 == parity
    bdiag = np.zeros((128, 48), dtype=ml_dtypes.bfloat16)
    for g in range(3):
        for par in range(2):
            for j in range(GRP):
                if (5 * g + j) % 2 != par:
                    continue
                for c in range(C1):
                    bdiag[22 * j + c, (2 * g + par) * 8 + (5 * g + j) // 2] = 1
    # single leftover slab (phase row 15 -> q=7, parity 1): [22, 8] col 7
    single = np.zeros((128, 8), dtype=ml_dtypes.bfloat16)
    for c in range(C1):
        single[c, 7] = 1
    cmat = np.concatenate([sel8, bdiag, single], axis=1)  # [128, 120]

    iotas = np.zeros((128, 2), dtype=np.float32)
    iotas[:, 0] = np.arange(128)
    iotas[:110, 1] = C0 + (np.arange(110) % C1)
    return cmat, iotas


def build_nc():
    nc = bacc.Bacc(None)
    x = nc.dram_tensor("x", [NUM_CLASSES, PIX_CORE], F32, kind="ExternalInput")
    tb = nc.dram_tensor("tb", [NSLAB, SLAB], BF16, kind="ExternalInput")
    cmat = nc.dram_tensor("cmat", [128, 120], BF16, kind="ExternalInput")
    iotas = nc.dram_tensor("iotas", [128, 2], F32, kind="ExternalInput")
    out = nc.dram_tensor("out", [128, OUT_COLS], F32, kind="ExternalOutput")

    dma = nc.gpsimd.dma_start  # SWDGE: no sync-wait-count limit

    with tile.TileContext(nc) as tc, ExitStack() as ctx:
        const_p = ctx.enter_context(tc.tile_pool(name="const", bufs=1))
        x0_p = ctx.enter_context(tc.tile_pool(name="x0", bufs=2))
        x1_p = ctx.enter_context(tc.tile_pool(name="x1", bufs=2))
        e0_p = ctx.enter_context(tc.tile_pool(name="e0", bufs=9))
        e1_p = ctx.enter_context(tc.tile_pool(name="e1", bufs=5))
        tbw_p = ctx.enter_context(tc.tile_pool(name="tbw", bufs=2))
        wf_p = ctx.enter_context(tc.tile_pool(name="wf", bufs=1))
        tw_p = ctx.enter_context(tc.tile_pool(name="tw", bufs=2))
        bc_p = ctx.enter_context(tc.tile_pool(name="bc", bufs=3))
        bc1_p = ctx.enter_context(tc.tile_pool(name="bc1", bufs=2))
        mask_p = ctx.enter_context(tc.tile_pool(name="mask", bufs=3))
        pp_p = ctx.enter_context(tc.tile_pool(name="pp", bufs=3))
        pm_p = ctx.enter_context(tc.tile_pool(name="pm", bufs=2))
        scr_p = ctx.enter_context(tc.tile_pool(name="scr", bufs=1))
        acc_p = ctx.enter_context(tc.tile_pool(name="acc", bufs=1))
        zp_p = ctx.enter_context(
            tc.tile_pool(name="zp", bufs=2, space=bass.MemorySpace.PSUM)
        )

        cm = const_p.tile([128, 120], BF16, tag="cm")
        dma(cm[:], cmat[:])
        io = const_p.tile([128, 2], F32, tag="io")
        dma(io[:], iotas[:])

        is0 = acc_p.tile([128, N_P_COLS], F32, tag="is0")
        xs0 = acc_p.tile([128, N_P_COLS], F32, tag="xs0")
        ys0 = acc_p.tile([128, N_P_COLS], F32, tag="ys0")
        is1 = acc_p.tile([110, N_C1_COLS], F32, tag="is1")
        xs1 = acc_p.tile([110, N_C1_COLS], F32, tag="xs1")
        ys1 = acc_p.tile([110, N_C1_COLS], F32, tag="ys1")
        for t in (is1, xs1, ys1):
            nc.vector.memset(t[:], 0.0)

        for ph in range(NPHASE):
            base = ph * PHASE_SLABS  # first slab of phase
            zp = zp_p.tile([NPAIR, 2 * SLAB], F32, tag="zp")

            # ---- stage A: load, exp, Z accumulation ----
            e0s = []
            for q in range(NPAIR):
                s = base + 2 * q
                x0 = x0_p.tile([128, 2 * SLAB], F32, tag="x0")
                dma(x0[:], x[0:128, s * SLAB:(s + 2) * SLAB])
                e0 = e0_p.tile([128, 2 * SLAB], BF16, tag="e0")
                nc.scalar.activation(e0[:], x0[:], AF.Exp)
                e0s.append(e0)
                lhsT = cm[:, q * 8:(q + 1) * 8]
                for h in range(2):
                    for j in range(2):
                        off = h * SLAB + j * 512
                        nc.tensor.matmul(
                            zp[:, off:off + 512],
                            lhsT,
                            e0[:, off:off + 512],
                            start=(q == 0),
                            stop=False,
                        )

            e1s = []
            for g in range(3):
                s = base + g * GRP
                x1 = x1_p.tile([110, SLAB], F32, tag="x1")
                # one DMA: dims (slab j, class c, pixel) all nonzero strides
                src = _ap(
                    x[C0:NUM_CLASSES, s * SLAB:(s + 1) * SLAB],
                    [[SLAB, GRP], [PIX_CORE, C1], [1, SLAB]],
                )
                dma(x1[:], src)
                e1 = e1_p.tile([110, SLAB], BF16, tag="e1")
                nc.scalar.activation(e1[:], x1[:], AF.Exp)
                e1s.append(e1)
                for par in range(2):
                    lhsT = cm[0:110, 64 + (2 * g + par) * 8:64 + (2 * g + par + 1) * 8]
                    for j in range(2):
                        nc.tensor.matmul(
                            zp[:, par * SLAB + j * 512:par * SLAB + (j + 1) * 512],
                            lhsT,
                            e1[:, j * 512:(j + 1) * 512],
                            start=False,
                            stop=(g == 2 and par == 0),
                        )
            # leftover slab (phase row 15)
            s15 = base + 15
            x1s = x1_p.tile([C1, SLAB], F32, tag="x1")
            dma(x1s[:], x[C0:NUM_CLASSES, s15 * SLAB:(s15 + 1) * SLAB])
            e1x = e1_p.tile([C1, SLAB], BF16, tag="e1")
            nc.scalar.activation(e1x[:], x1s[:], AF.Exp)
            lhsT = cm[0:C1, 112:120]
            for j in range(2):
                nc.tensor.matmul(
                    zp[:, SLAB + j * 512:SLAB + (j + 1) * 512],
                    lhsT,
                    e1x[:, j * 512:(j + 1) * 512],
                    start=False,
                    stop=True,
                )

            # ---- stage W ----
            tbp = tbw_p.tile([PHASE_SLABS, SLAB], BF16, tag="tbp")
            dma(tbp[:], tb[base:base + PHASE_SLABS, :])
            wf = wf_p.tile([NPAIR, 2 * SLAB], F32, tag="wf")
            wscr = wf_p.tile([NPAIR, 2 * SLAB], F32, tag="wscr")
            nc.vector.reciprocal_approx_accurate(out=wf[:], in_=zp[:], scratch=wscr[:])
            wb = tbw_p.tile([NPAIR, 2 * SLAB], BF16, tag="wb")
            nc.vector.tensor_copy(wb[:], wf[:])

            # TW pair rows: [8, 4096] row q = [T(2q) | T(2q+1) | W(2q) | W(2q+1)]
            twt = tw_p.tile([NPAIR, 4 * SLAB], BF16, tag="twt")
            # T halves straight from DRAM (pairs fold: tb rows 2q,2q+1 -> row q)
            dma(
                twt[:, 0:2 * SLAB],
                _ap(tb[base:base + PHASE_SLABS, :],
                    [[2 * SLAB, NPAIR], [1, 2 * SLAB]]),
            )
            # W half: wb is already pair-shaped
            dma(twt[:, 2 * SLAB:4 * SLAB], wb[:])

            # ---- stage B ----
            def stageB(e_t, parts, tcol, wcol, iota_ap, acc_i, acc_x, acc_y,
                       col, fd):
                mask = mask_p.tile([128, 2 * SLAB], BF16, tag="mask")
                m = mask[0:parts, 0:fd]
                nc.vector.tensor_scalar(
                    m, tcol, iota_ap, 0.0, OP.is_equal, op1=OP.add,
                    accum_out=acc_y[0:parts, col:col + 1],
                )
                p = pp_p.tile([128, 2 * SLAB], BF16, tag="pp")
                pt = p[0:parts, 0:fd]
                nc.vector.tensor_tensor(pt, e_t, wcol, OP.mult)
                pm = pm_p.tile([128, 2 * SLAB], BF16, tag="pm")
                pmt = pm[0:parts, 0:fd]
                nc.vector.tensor_tensor(pmt, pt, m, OP.mult)
                scr = scr_p.tile([128, 2 * SLAB], BF16, tag="scr")
                nc.vector.tensor_scalar(
                    scr[0:parts, 0:fd], pmt, 1.0, 0.0, OP.mult, op1=OP.add,
                    accum_out=acc_i[0:parts, col:col + 1],
                )
                scra = scr_p.tile([128, 2 * SLAB], BF16, tag="scra")
                nc.scalar.activation(
                    scra[0:parts, 0:fd], pt, AF.Square,
                    accum_out=acc_x[0:parts, col:col + 1],
                )

            for q in range(NPAIR):
                twbc = bc_p.tile([128, 4 * SLAB], BF16, tag="twbc")
                dma(twbc[:], _bcast_src(twt[q:q + 1, :], 128))
                stageB(
                    e0s[q][:], 128,
                    twbc[:, 0:2 * SLAB], twbc[:, 2 * SLAB:4 * SLAB],
                    io[:, 0:1], is0, xs0, ys0, ph * NPAIR + q, 2 * SLAB,
                )

            for g in range(4):
                parts = 110 if g < 3 else C1
                nsl = GRP if g < 3 else 1
                r0 = g * GRP
                tbc = bc1_p.tile([110, SLAB], BF16, tag="tbc1")
                wbc = bc1_p.tile([110, SLAB], BF16, tag="wbc1")
                for j in range(nsl):
                    r = r0 + j
                    dma(
                        tbc[22 * j:22 * j + 22, :],
                        _bcast_src(tbp[r:r + 1, :], C1),
                    )
                    dma(
                        wbc[22 * j:22 * j + 22, :],
                        _bcast_src(
                            wb[r // 2:r // 2 + 1,
                               (r % 2) * SLAB:(r % 2 + 1) * SLAB], C1),
                    )
                e_t = (e1s[g][:] if g < 3 else e1x[:])
                stageB(
                    e_t, parts, tbc[0:parts, :], wbc[0:parts, :],
                    io[0:parts, 1:2], is1, xs1, ys1, ph * 4 + g, SLAB,
                )

        c0, c1 = N_P_COLS, N_C1_COLS
        dma(out[:, 0:c0], is0[:])
        dma(out[:, c0:2 * c0], xs0[:])
        dma(out[:, 2 * c0:3 * c0], ys0[:])
        b0 = 3 * c0
        dma(out[0:110, b0:b0 + c1], is1[:])
        dma(out[0:110, b0 + c1:b0 + 2 * c1], xs1[:])
        dma(out[0:110, b0 + 2 * c1:b0 + 3 * c1], ys1[:])

    nc.finalize()
    return nc


def make_in_maps(inputs, targets):
    x = np.asarray(inputs, dtype=np.float32).reshape(4, NUM_CLASSES, 512 * 512)
    t = np.asarray(targets)
    t = np.where(t == 255, 0, t).reshape(4, 512 * 512)
    cmat, iotas = _build_consts()
    in_maps = []
    for k in range(N_CORES):
        b, po = k // 2, (k % 2) * PIX_CORE
        xk = np.ascontiguousarray(x[b, :, po:po + PIX_CORE])
        tk = t[b, po:po + PIX_CORE].astype(ml_dtypes.bfloat16).reshape(NSLAB, SLAB)
        in_maps.append({"x": xk, "tb": tk, "cmat": cmat, "iotas": iotas})
    return in_maps


def decode_outputs(outs):
    """outs: list of 8 [128, OUT_COLS] f32 arrays -> (loss, dice)."""
    IS = np.zeros(NUM_CLASSES, dtype=np.float64)
    XS = np.zeros(NUM_CLASSES, dtype=np.float64)
    YS = np.zeros(NUM_CLASSES, dtype=np.float64)
    c0, c1 = N_P_COLS, N_C1_COLS
    cols_g = [c for c in range(c1) if c % 4 != 3]
    cols_s = [c for c in range(c1) if c % 4 == 3]
    b0 = 3 * c0
    for o in outs:
        o = np.asarray(o, dtype=np.float64)
        IS[:C0] += o[:, 0:c0].sum(axis=1)
        XS[:C0] += o[:, c0:2 * c0].sum(axis=1)
        YS[:C0] += o[:, 2 * c0:3 * c0].sum(axis=1)
        for dst, bb in ((IS, b0), (XS, b0 + c1), (YS, b0 + 2 * c1)):
            blk = o[0:110, bb:bb + c1]
            tmp = blk[:, cols_g].reshape(GRP, C1, len(cols_g)).sum(axis=(0, 2))
            dst[C0:] += tmp + blk[0:C1, cols_s].sum(axis=1)
    dice = (2.0 * IS + SMOOTH) / (XS + YS + SMOOTH)
    loss = np.mean(1.0 - dice)
    return np.float32(loss), dice.astype(np.float32)


_NC_CACHE = {}


def _get_nc():
    if "nc" not in _NC_CACHE:
        _NC_CACHE["nc"] = build_nc()
    return _NC_CACHE["nc"]


def kernel(inputs, targets):
    from concourse.bass_utils import run_bass_kernel_spmd

    nc = _get_nc()
    in_maps = make_in_maps(inputs, targets)
    res = run_bass_kernel_spmd(nc, in_maps, list(range(N_CORES)))
    outs = [res.results[i]["out"] for i in range(N_CORES)]
    return decode_outputs(outs)
